# revision 13
# baseline (speedup 1.0000x reference)
"""ChebyKAN layer kernel for TRN2 (8 NeuronCores, SPMD data-parallel over B).

y[b,o] = sum_{i,d} cos(d*arccos(tanh(x[b,i]))) * C[i,o,d]
       = sum_d T_d(tanh(x)) @ C[:,:,d]      (Chebyshev recurrence, exact)

v12: degrees 1-3 plus 3/4 of degree 4 (k-pairs 0-2) run as fp8-e4m3
DoubleRow matmuls in the FIRST PSUM accumulation group; the rest of
degree 4 and degrees 5-8 run as bf16 matmuls in a second group.
Host error-sim (bit-faithful; v11 predicted 1.9353e-2, HW measured
1.936e-2) puts this mix at 1.935e-2 vs the 2e-2 gate.

Critical-path engineering on top of v11 (trace-driven):
 - fp8 basis tiles are k-major 4D [128, KP, 2, bc]: the DoubleRow lhsT
   takes strided pair APs directly, so dtype conversion is two
   contiguous half-copies instead of eight slice-copies.
 - the recurrence runs in k-halves with the multiply on GpSimdE and the
   in-place scalar_tensor_tensor on VectorE, so each degree's basis
   half is ready ~6us after the previous one's chain instead of ~14us
   (the v11 trace showed a 6.1us PE gap per fp8 degree handoff).
 - tanh + degree-1 conversion for chunk c are emitted during chunk
   c-1's degree-8 phase (ScalarE is idle there), so chunk boundaries
   hand off without a basis stall.
 - DMA is split across both rings: sync carries x0/x1 + fp8 weights +
   bf16 degrees 6/8, the gpsimd SWDGE carries x2+ + wb4 + bf16 degrees
   5/7 + outputs (the v11 sync ring saturated and cost a 4.4us stall
   at group-B start).
 - the HAM warmup bridge (memset tile, no DMA dependency) is sized to
   the measured cold-MM rate (~430-630ns at K=4/8).
"""
import numpy as np
import ml_dtypes
from contextlib import ExitStack

import concourse.bass as bass
import concourse.tile as tile
from concourse import bacc, mybir
from concourse.bass_utils import run_bass_kernel_spmd

F32 = mybir.dt.float32
BF16 = mybir.dt.bfloat16
FP8 = mybir.dt.float8e4
DR = mybir.MatmulPerfMode.DoubleRow
TANH = mybir.ActivationFunctionType.Tanh
COPY = mybir.ActivationFunctionType.Copy
MULT = mybir.AluOpType.mult
SUBTRACT = mybir.AluOpType.subtract
ADD = mybir.AluOpType.add

B, I, O, DEG = 16384, 1024, 1024, 8
N_CORES = 8
B_SHARD = B // N_CORES
W8_SCALE = 2.0 ** 14       # host pre-scale for fp8 weights
N_WARM = 34                # memset-tile warmup matmuls bridging DMA startup


def _cfg(I_):
    KT = I_ // 128
    KP = KT // 2
    d4kp = (3 * KP) // 4      # k-pairs of degree 4 in fp8 (3 of 4 at full size)
    nb4 = KT - 2 * d4kp       # k-slices of degree 4 in bf16
    return KT, KP, d4kp, nb4


def build_nc(I_=I, O_=O, b_shard=B_SHARD, b_chunk=512):
    """Build the per-core Bass program (SPMD: same program, sharded x)."""
    KT, KP, d4kp, nb4 = _cfg(I_)
    NT8 = 3 * KP + d4kp     # fp8 pair-tiles per chunk
    MT = b_chunk // 128     # output-row tiles per chunk (PSUM partition dim)
    OHT = O_ // 512         # output-col halves per chunk (PSUM free dim)
    n_chunks = b_shard // b_chunk
    FD = KT * b_chunk       # free dim of basis tiles (k-major concat)
    NH = 2 if KT >= 4 else 1   # k-halves for the recurrence pipeline
    assert MT * OHT <= 8 and KT % 2 == 0

    nc = bacc.Bacc("TRN2", target_bir_lowering=False, debug=False)
    xT = nc.dram_tensor("xT", [I_, b_shard], F32, kind="ExternalInput").ap()
    w8p = nc.dram_tensor("w8p", [NT8, 128, 2, O_], FP8, kind="ExternalInput").ap()
    wb4 = nc.dram_tensor("wb4", [nb4, 128, O_], BF16, kind="ExternalInput").ap()
    whi = nc.dram_tensor("whi", [4, I_, O_], BF16, kind="ExternalInput").ap()
    biasrep = nc.dram_tensor("biasrep", [128, O_], F32, kind="ExternalInput").ap()
    y = nc.dram_tensor("y", [b_shard, O_], F32, kind="ExternalOutput").ap()

    with tile.TileContext(nc) as tc, ExitStack() as ctx:
        const_pool = ctx.enter_context(tc.tile_pool(name="const", bufs=1))
        x_pool = ctx.enter_context(tc.tile_pool(name="x", bufs=2))
        chain_pool = ctx.enter_context(tc.tile_pool(name="chain", bufs=1))
        bb_pool = ctx.enter_context(tc.tile_pool(name="bb", bufs=2))
        t8_pool = ctx.enter_context(tc.tile_pool(name="t8", bufs=2))
        t8b_pool = ctx.enter_context(tc.tile_pool(name="t8b", bufs=1))
        w_pool = ctx.enter_context(tc.tile_pool(name="w", bufs=2))
        w8_pool = ctx.enter_context(tc.tile_pool(name="w8", bufs=2))
        wb4_pool = ctx.enter_context(tc.tile_pool(name="wb4", bufs=1))
        stage_pool = ctx.enter_context(tc.tile_pool(name="stage", bufs=1))
        warm_pool = ctx.enter_context(tc.tile_pool(name="warm", bufs=1))
        psum_pool = ctx.enter_context(tc.tile_pool(name="psum", bufs=1, space="PSUM"))

        def load_w8(c, d, eng=None):
            """fp8 pair-tiles [128, 2, O] for fp8 degree d (host-paired)."""
            nkp = d4kp if d == 4 else KP
            ws = []
            for kp in range(nkp):
                f = 3 * KP + kp if d == 4 else (d - 1) * KP + kp
                wk = w8_pool.tile([128, 2, O_], FP8, tag=f"w8_{kp}",
                                  name=f"w8d{d}_{kp}_c{c}")
                (eng or nc.sync).dma_start(out=wk[:], in_=w8p[f])
                ws.append(wk)
            return ws

        def load_w(c, d, eng=None):
            """Per-k bf16 weight tiles for degree d (5..8)."""
            ws = []
            for k in range(KT):
                wk = w_pool.tile([128, O_], BF16, tag=f"w{k}",
                                 name=f"w{d}k{k}_c{c}")
                (eng or nc.sync).dma_start(
                    out=wk[:], in_=whi[d - 5, k * 128:(k + 1) * 128, :])
                ws.append(wk)
            return ws

        def load_wb4(c):
            ws = []
            for j in range(nb4):
                wk = wb4_pool.tile([128, O_], BF16, tag=f"wb4_{j}",
                                   name=f"wb4_{j}_c{c}")
                nc.gpsimd.dma_start(out=wk[:], in_=wb4[j])
                ws.append(wk)
            return ws

        def prep_head(c):
            """x DMAs + tanh + degree-1 fp8 conversion for chunk c.  Called
            during chunk c-1's degree-8 phase so ScalarE does this in its
            idle tail window and chunk c's first matmuls start immediately."""
            xs = []
            for k in range(KT):
                xk = x_pool.tile([128, b_chunk], F32, tag=f"x{k}",
                                 name=f"x{k}_c{c}")
                (nc.sync if k < 2 else nc.gpsimd).dma_start(
                    out=xk[:],
                    in_=xT[k * 128:(k + 1) * 128,
                           c * b_chunk:(c + 1) * b_chunk])
                xs.append(xk)
            t1 = chain_pool.tile([128, FD], F32, tag="t1", name=f"t1_c{c}")
            t8_1 = t8_pool.tile([128, KP, 2, b_chunk], FP8, tag="t8",
                                name=f"t8d1_c{c}")
            for k in range(KT):
                nc.scalar.activation(t1[:, k * b_chunk:(k + 1) * b_chunk],
                                     xs[k][:], TANH)
                if k % 2 == 1:
                    # k-major fp8 copy of the finished k-pair (dst free dims
                    # (2, bc) iterate the same contiguous run as the src)
                    q = (k - 1) * b_chunk
                    nc.scalar.activation(t8_1[:, k // 2, :, :],
                                         t1[:, q:q + 2 * b_chunk], COPY)
            return xs, t1, t8_1

        # HAM warmup bridge on a memset tile: no DMA dependency, so the PE
        # clock ramps from ~0 and stays at 8/8 through the DMA-bound startup
        # window.  Writes PSUM bank 0, reset by the first start=True matmul.
        warm_t = warm_pool.tile([128, 640], BF16, tag="warm")
        nc.vector.memset(warm_t[:], 0.0)
        warm_ps = psum_pool.tile([128, 512], F32, tag="ps0_0", name="warm_ps")
        for i in range(N_WARM):
            nc.tensor.matmul(warm_ps[:], warm_t[:, 0:128],
                             warm_t[:, 128:640], start=True, stop=True)

        # x leads both DMA rings (tanh gates everything); bias rides last
        # (first needed at the degree-4 close, ~45us in)
        x_next, t1_next, t81_next = prep_head(0)
        w8d1_next = load_w8(0, 1)
        bias_t = const_pool.tile([128, O_], F32, tag="biasrep")
        nc.gpsimd.dma_start(out=bias_t[:], in_=biasrep)

        for c in range(n_chunks):
            b0 = c * b_chunk
            x_t, t1, t8_1 = x_next, t1_next, t81_next

            rings = [chain_pool.tile([128, FD], F32, tag=f"r{r}", name=f"r{r}_c{c}")
                     for r in range(3)]
            # stage doubles as group-1 staging and group-2 eviction buffer
            stage = stage_pool.tile([128, MT * OHT * 512], F32, tag="stage",
                                    name=f"st_c{c}")
            ps = [[psum_pool.tile([128, 512], F32, tag=f"ps{m}_{oh}",
                                  name=f"ps{m}_{oh}_c{c}")
                   for oh in range(OHT)] for m in range(MT)]

            wb4_t = load_wb4(c)
            w_d5 = load_w(c, 5, eng=nc.gpsimd)

            def rec_half(cur, prev1, prev2, h):
                """T_next = 2*t1*prev1 - prev2 on k-half h (DVE multiply +
                in-place scalar_tensor_tensor)."""
                hs = slice(h * (FD // NH), (h + 1) * (FD // NH))
                nc.vector.tensor_tensor(cur[:, hs], t1[:, hs], prev1[:, hs], MULT)
                nc.vector.scalar_tensor_tensor(
                    cur[:, hs], cur[:, hs], 2.0, prev2[:, hs], MULT, SUBTRACT)

            def dbl_half(cur, src, h):
                """T_2n = 2*T_n^2 - 1 on k-half h: Square on ScalarE (keeps
                the DVE free in the fp8 window) + in-place DVE axpb."""
                hs = slice(h * (FD // NH), (h + 1) * (FD // NH))
                nc.scalar.activation(cur[:, hs], src[:, hs],
                                     mybir.ActivationFunctionType.Square)
                nc.vector.tensor_scalar(cur[:, hs], cur[:, hs],
                                        2.0, -1.0, MULT, ADD)

            def conv8_half(dst, src, h, nkp):
                """fp8 copy of k-half h into the k-major 4D pair tile."""
                kp0, kp1 = h * (KT // (2 * NH)), min(nkp, (h + 1) * (KT // (2 * NH)))
                if kp0 >= kp1:
                    return
                nc.scalar.activation(
                    dst[:, kp0:kp1, :, :],
                    src[:, kp0 * 2 * b_chunk:kp1 * 2 * b_chunk], COPY)

            # ---- group A: fp8 DoubleRow degrees 1..3 (+ partial 4) ----
            t8_t = {1: t8_1}
            w8_d = {1: w8d1_next, 2: load_w8(c, 2)}
            t_prev2, t_prev1 = None, t1
            for d in (1, 2, 3):
                if d == 1:
                    w8_d[3] = load_w8(c, 3)
                elif d == 2 and d4kp > 0:
                    w8_d[4] = load_w8(c, 4)
                # recurrence + conversion for the NEXT degree, half by half
                nd = d + 1
                cur = rings[(nd - 2) % 3]
                t8_n = None
                if nd <= 3:
                    t8_n = t8_pool.tile([128, KP, 2, b_chunk], FP8, tag="t8",
                                        name=f"t8d{nd}_c{c}")
                elif d4kp > 0:
                    t8_n = t8b_pool.tile([128, d4kp, 2, b_chunk], FP8,
                                         tag="t8b", name=f"t8d4_c{c}")
                for h in range(NH):
                    if nd == 2:
                        dbl_half(cur, t1, h)          # T2 = 2*T1^2 - 1
                    elif nd == 4:
                        dbl_half(cur, rings[0], h)    # T4 = 2*T2^2 - 1
                    else:
                        rec_half(cur, t_prev1, t_prev2, h)
                    if t8_n is not None:
                        conv8_half(t8_n, cur, h, KP if nd <= 3 else d4kp)
                t_prev2, t_prev1 = t_prev1, cur
                if t8_n is not None:
                    t8_t[nd] = t8_n
                # this degree's matmuls (k-outer: stream as halves finish)
                closing = (d == 3 and d4kp == 0)
                if not closing:
                    for kp in range(KP):
                        for m in range(MT):
                            lhsT = t8_t[d][:, kp, :, m * 128:(m + 1) * 128]
                            for oh in range(OHT):
                                nc.tensor.matmul(
                                    ps[m][oh][:], lhsT,
                                    w8_d[d][kp][:, :, oh * 512:(oh + 1) * 512],
                                    start=(d == 1 and kp == 0), stop=False,
                                    perf_mode=DR)

            # degree 4 bf16 leftover conversion
            cur4 = t_prev1
            tb4 = None
            if nb4 > 0:
                tb4 = bb_pool.tile([128, nb4 * b_chunk], BF16, tag="tb4",
                                   name=f"tb4_c{c}")
                nc.scalar.activation(
                    tb4[:], cur4[:, 2 * d4kp * b_chunk:KT * b_chunk], COPY)

            # close group A per bank (k-contiguous) and stage the rescaled
            # partial + bias; banks free up one by one for the bf16 group
            close_d = 4 if d4kp > 0 else 3
            close_t8 = t8_t[close_d]
            close_w8 = w8_d[close_d]
            close_kp = KP if close_d == 3 else d4kp
            for m in range(MT):
                for oh in range(OHT):
                    for kp in range(close_kp):
                        lhsT = close_t8[:, kp, :, m * 128:(m + 1) * 128]
                        nc.tensor.matmul(
                            ps[m][oh][:], lhsT,
                            close_w8[kp][:, :, oh * 512:(oh + 1) * 512],
                            start=False, stop=(kp == close_kp - 1),
                            perf_mode=DR)
                    so = (m * OHT + oh) * 512
                    nc.vector.scalar_tensor_tensor(
                        stage[:, so:so + 512], ps[m][oh][:],
                        1.0 / W8_SCALE, bias_t[:, oh * 512:(oh + 1) * 512],
                        MULT, ADD)

            # ---- group B: bf16 degrees (rest of 4, then 5..8) ----
            if nb4 > 0:
                for j in range(nb4):
                    for m in range(MT):
                        lhsT = tb4[:, j * b_chunk + m * 128:
                                   j * b_chunk + (m + 1) * 128]
                        for oh in range(OHT):
                            nc.tensor.matmul(
                                ps[m][oh][:], lhsT,
                                wb4_t[j][:, oh * 512:(oh + 1) * 512],
                                start=(j == 0), stop=False)

            w_next_hi = w_d5
            for d in range(5, DEG + 1):
                w_t = w_next_hi
                # recurrence + bf16 conversion for this degree
                cur = rings[(d - 2) % 3]
                for h in range(NH):
                    rec_half(cur, t_prev1, t_prev2, h)
                t_prev2, t_prev1 = t_prev1, cur
                tb = bb_pool.tile([128, FD], BF16, tag="bb", name=f"tb{d}_c{c}")
                for q in range(4):
                    qs = slice(q * (FD // 4), (q + 1) * (FD // 4))
                    nc.scalar.activation(tb[:, qs], cur[:, qs], COPY)
                if d < DEG:
                    # degrees 6/8 ride sync, degree 7 rides the SWDGE
                    w_next_hi = load_w(c, d + 1,
                                       eng=nc.gpsimd if d == 6 else nc.sync)

                if d < DEG:
                    start_b = (d == 5 and nb4 == 0)
                    for k in range(KT):
                        for m in range(MT):
                            lhsT = tb[:, k * b_chunk + m * 128:
                                      k * b_chunk + (m + 1) * 128]
                            for oh in range(OHT):
                                nc.tensor.matmul(
                                    ps[m][oh][:], lhsT,
                                    w_t[k][:, oh * 512:(oh + 1) * 512],
                                    start=(start_b and k == 0), stop=False)
                else:
                    if c + 1 < n_chunks:
                        x_next, t1_next, t81_next = prep_head(c + 1)
                        w8d1_next = load_w8(c + 1, 1)
                    # close group B per bank; combine with staged group-A
                    # partial in-place and DMA out, bank by bank
                    for m in range(MT):
                        for oh in range(OHT):
                            for k in range(KT):
                                lhsT = tb[:, k * b_chunk + m * 128:
                                          k * b_chunk + (m + 1) * 128]
                                nc.tensor.matmul(
                                    ps[m][oh][:], lhsT,
                                    w_t[k][:, oh * 512:(oh + 1) * 512],
                                    start=False, stop=(k == KT - 1))
                            so = (m * OHT + oh) * 512
                            nc.vector.tensor_tensor(
                                stage[:, so:so + 512], ps[m][oh][:],
                                stage[:, so:so + 512], ADD)
                            nc.gpsimd.dma_start(
                                out=y[b0 + m * 128: b0 + (m + 1) * 128,
                                      oh * 512:(oh + 1) * 512],
                                in_=stage[:, so:so + 512])
    nc.compile()
    return nc


_NC_CACHE = {}


def _install_ntff_hook():
    """Provide antenv.axon_hooks (missing in this image) so trace=True works."""
    import sys
    import types
    if "antenv.axon_hooks" in sys.modules:
        return
    hook = None
    try:
        from trn_agent_boot.trn_boot import _ntff_profile_via_ctypes
        hook = _ntff_profile_via_ctypes("/opt/axon/libaxon_pjrt.so")
    except Exception:
        pass
    mod = types.ModuleType("antenv.axon_hooks")
    mod.get_axon_ntff_profile_hook = lambda: hook
    sys.modules["antenv.axon_hooks"] = mod
    # no remote artifact bucket in this container
    import concourse.bass_utils as _bu
    _bu.upload_artifacts = lambda tmpdir: tmpdir


def _prep_inputs(x, cheby_coeffs, b_shard=B_SHARD, n_cores=N_CORES):
    coeffs = np.asarray(cheby_coeffs, dtype=np.float32)
    I_ = coeffs.shape[0]
    O_ = coeffs.shape[1]
    KT, KP, d4kp, nb4 = _cfg(I_)
    wmoved = np.moveaxis(coeffs[:, :, 1:], 2, 0)      # (DEG, I, O)

    # fp8 pair-tiles: degrees 1..3 all k-pairs, degree 4 first d4kp pairs.
    # layout [tile, partition, j, o] matches the [128, 2, O] SBUF tile.
    NT8 = 3 * KP + d4kp
    w8p = np.empty((NT8, 128, 2, O_), dtype=np.float32)
    for d in (1, 2, 3, 4):
        nkp = d4kp if d == 4 else KP
        for kp in range(nkp):
            f = 3 * KP + kp if d == 4 else (d - 1) * KP + kp
            for j in range(2):
                sl = wmoved[d - 1, (2 * kp + j) * 128:(2 * kp + j + 1) * 128, :]
                w8p[f, :, j, :] = sl
    w8p = np.ascontiguousarray(w8p * W8_SCALE).astype(ml_dtypes.float8_e4m3)

    wb4 = np.ascontiguousarray(
        wmoved[3, 2 * d4kp * 128:, :].reshape(nb4, 128, O_)
    ).astype(ml_dtypes.bfloat16)
    whi = np.ascontiguousarray(wmoved[4:]).astype(ml_dtypes.bfloat16)
    bias = coeffs[:, :, 0].astype(np.float64).sum(axis=0).astype(np.float32)
    biasrep = np.ascontiguousarray(np.broadcast_to(bias, (128, O_)))
    xT = np.asarray(x, dtype=np.float32).T  # (I, B)
    in_maps = []
    for c in range(n_cores):
        in_maps.append({
            "xT": np.ascontiguousarray(xT[:, c * b_shard:(c + 1) * b_shard]),
            "w8p": w8p,
            "wb4": wb4,
            "whi": whi,
            "biasrep": biasrep,
        })
    return in_maps


def kernel(x: np.ndarray, cheby_coeffs: np.ndarray, _trace: bool = False):
    assert x.shape == (B, I) and cheby_coeffs.shape == (I, O, DEG + 1)
    if _trace:
        _install_ntff_hook()
    if "nc" not in _NC_CACHE:
        _NC_CACHE["nc"] = build_nc()
    nc = _NC_CACHE["nc"]

    in_maps = _prep_inputs(x, cheby_coeffs)
    res = run_bass_kernel_spmd(nc, in_maps, list(range(N_CORES)), trace=_trace)
    out = np.concatenate([res.results[c]["y"] for c in range(N_CORES)], axis=0)
    if _trace:
        return out, res
    return out


# revision 15
# speedup vs baseline: 1.0210x; 1.0210x over previous
"""ChebyKAN layer kernel for TRN2 (8 NeuronCores, SPMD data-parallel over B).

y[b,o] = sum_{i,d} cos(d*arccos(tanh(x[b,i]))) * C[i,o,d]
       = sum_d T_d(tanh(x)) @ C[:,:,d]      (Chebyshev recurrence, exact)

v12: degrees 1-3 plus 3/4 of degree 4 (k-pairs 0-2) run as fp8-e4m3
DoubleRow matmuls in the FIRST PSUM accumulation group; the rest of
degree 4 and degrees 5-8 run as bf16 matmuls in a second group.
Host error-sim (bit-faithful; v11 predicted 1.9353e-2, HW measured
1.936e-2) puts this mix at 1.935e-2 vs the 2e-2 gate.

Critical-path engineering on top of v11 (trace-driven):
 - fp8 basis tiles are k-major 4D [128, KP, 2, bc]: the DoubleRow lhsT
   takes strided pair APs directly, so dtype conversion is two
   contiguous half-copies instead of eight slice-copies.
 - the recurrence runs in k-halves with the multiply on GpSimdE and the
   in-place scalar_tensor_tensor on VectorE, so each degree's basis
   half is ready ~6us after the previous one's chain instead of ~14us
   (the v11 trace showed a 6.1us PE gap per fp8 degree handoff).
 - tanh + degree-1 conversion for chunk c are emitted during chunk
   c-1's degree-8 phase (ScalarE is idle there), so chunk boundaries
   hand off without a basis stall.
 - DMA is split across both rings: sync carries x0/x1 + fp8 weights +
   bf16 degrees 6/8, the gpsimd SWDGE carries x2+ + wb4 + bf16 degrees
   5/7 + outputs (the v11 sync ring saturated and cost a 4.4us stall
   at group-B start).
 - the HAM warmup bridge (memset tile, no DMA dependency) is sized to
   the measured cold-MM rate (~430-630ns at K=4/8).
"""
import numpy as np
import ml_dtypes
from contextlib import ExitStack

import concourse.bass as bass
import concourse.tile as tile
from concourse import bacc, mybir
from concourse.bass_utils import run_bass_kernel_spmd

F32 = mybir.dt.float32
BF16 = mybir.dt.bfloat16
FP8 = mybir.dt.float8e4
DR = mybir.MatmulPerfMode.DoubleRow
TANH = mybir.ActivationFunctionType.Tanh
COPY = mybir.ActivationFunctionType.Copy
MULT = mybir.AluOpType.mult
SUBTRACT = mybir.AluOpType.subtract
ADD = mybir.AluOpType.add

B, I, O, DEG = 16384, 1024, 1024, 8
N_CORES = 8
B_SHARD = B // N_CORES
W8_SCALE = 2.0 ** 14       # host pre-scale for fp8 weights
N_WARM = 34                # memset-tile warmup matmuls bridging DMA startup


def _cfg(I_):
    KT = I_ // 128
    KP = KT // 2
    d4kp = (3 * KP) // 4      # k-pairs of degree 4 in fp8 (3 of 4 at full size)
    nb4 = KT - 2 * d4kp       # k-slices of degree 4 in bf16
    return KT, KP, d4kp, nb4


def build_nc(I_=I, O_=O, b_shard=B_SHARD, b_chunk=512):
    """Build the per-core Bass program (SPMD: same program, sharded x)."""
    KT, KP, d4kp, nb4 = _cfg(I_)
    NT8 = 3 * KP + d4kp     # fp8 pair-tiles per chunk
    MT = b_chunk // 128     # output-row tiles per chunk (PSUM partition dim)
    OHT = O_ // 512         # output-col halves per chunk (PSUM free dim)
    n_chunks = b_shard // b_chunk
    FD = KT * b_chunk       # free dim of basis tiles (k-major concat)
    NH = 2 if KT >= 4 else 1   # k-halves for the recurrence pipeline
    assert MT * OHT <= 8 and KT % 2 == 0

    nc = bacc.Bacc("TRN2", target_bir_lowering=False, debug=False)
    xT = nc.dram_tensor("xT", [I_, b_shard], F32, kind="ExternalInput").ap()
    w8p = nc.dram_tensor("w8p", [NT8, 128, 2, O_], FP8, kind="ExternalInput").ap()
    wb4 = nc.dram_tensor("wb4", [nb4, 128, O_], BF16, kind="ExternalInput").ap()
    whi = nc.dram_tensor("whi", [4, I_, O_], BF16, kind="ExternalInput").ap()
    biasrep = nc.dram_tensor("biasrep", [128, O_], F32, kind="ExternalInput").ap()
    y = nc.dram_tensor("y", [b_shard, O_], F32, kind="ExternalOutput").ap()

    with tile.TileContext(nc) as tc, ExitStack() as ctx:
        const_pool = ctx.enter_context(tc.tile_pool(name="const", bufs=1))
        x_pool = ctx.enter_context(tc.tile_pool(name="x", bufs=2))
        chain_pool = ctx.enter_context(tc.tile_pool(name="chain", bufs=1))
        bb_pool = ctx.enter_context(tc.tile_pool(name="bb", bufs=2))
        t8_pool = ctx.enter_context(tc.tile_pool(name="t8", bufs=2))
        t8b_pool = ctx.enter_context(tc.tile_pool(name="t8b", bufs=1))
        w_pool = ctx.enter_context(tc.tile_pool(name="w", bufs=2))
        w8_pool = ctx.enter_context(tc.tile_pool(name="w8", bufs=2))
        wb4_pool = ctx.enter_context(tc.tile_pool(name="wb4", bufs=1))
        stage_pool = ctx.enter_context(tc.tile_pool(name="stage", bufs=1))
        warm_pool = ctx.enter_context(tc.tile_pool(name="warm", bufs=1))
        psum_pool = ctx.enter_context(tc.tile_pool(name="psum", bufs=1, space="PSUM"))

        def load_w8(c, d, eng=None):
            """fp8 pair-tiles [128, 2, O] for fp8 degree d (host-paired)."""
            nkp = d4kp if d == 4 else KP
            ws = []
            for kp in range(nkp):
                f = 3 * KP + kp if d == 4 else (d - 1) * KP + kp
                wk = w8_pool.tile([128, 2, O_], FP8, tag=f"w8_{kp}",
                                  name=f"w8d{d}_{kp}_c{c}")
                (eng or nc.sync).dma_start(out=wk[:], in_=w8p[f])
                ws.append(wk)
            return ws

        def load_w(c, d, eng=None):
            """Per-k bf16 weight tiles for degree d (5..8)."""
            ws = []
            for k in range(KT):
                wk = w_pool.tile([128, O_], BF16, tag=f"w{k}",
                                 name=f"w{d}k{k}_c{c}")
                (eng or nc.sync).dma_start(
                    out=wk[:], in_=whi[d - 5, k * 128:(k + 1) * 128, :])
                ws.append(wk)
            return ws

        def load_wb4(c):
            ws = []
            for j in range(nb4):
                wk = wb4_pool.tile([128, O_], BF16, tag=f"wb4_{j}",
                                   name=f"wb4_{j}_c{c}")
                nc.gpsimd.dma_start(out=wk[:], in_=wb4[j])
                ws.append(wk)
            return ws

        def prep_head(c):
            """x DMAs + tanh + degree-1 fp8 conversion for chunk c.  Called
            during chunk c-1's degree-8 phase so ScalarE does this in its
            idle tail window and chunk c's first matmuls start immediately."""
            xs = []
            for k in range(KT):
                xk = x_pool.tile([128, b_chunk], F32, tag=f"x{k}",
                                 name=f"x{k}_c{c}")
                (nc.sync if k < 2 else nc.gpsimd).dma_start(
                    out=xk[:],
                    in_=xT[k * 128:(k + 1) * 128,
                           c * b_chunk:(c + 1) * b_chunk])
                xs.append(xk)
            t1 = chain_pool.tile([128, FD], F32, tag="t1", name=f"t1_c{c}")
            t8_1 = t8_pool.tile([128, KP, 2, b_chunk], FP8, tag="t8",
                                name=f"t8d1_c{c}")
            for k in range(KT):
                nc.scalar.activation(t1[:, k * b_chunk:(k + 1) * b_chunk],
                                     xs[k][:], TANH)
                if k % 2 == 1:
                    # k-major fp8 copy of the finished k-pair (dst free dims
                    # (2, bc) iterate the same contiguous run as the src)
                    q = (k - 1) * b_chunk
                    nc.scalar.activation(t8_1[:, k // 2, :, :],
                                         t1[:, q:q + 2 * b_chunk], COPY)
            return xs, t1, t8_1

        # HAM warmup bridge on a memset tile: no DMA dependency, so the PE
        # clock ramps from ~0 and stays at 8/8 through the DMA-bound startup
        # window.  Writes PSUM bank 0, reset by the first start=True matmul.
        warm_t = warm_pool.tile([128, 640], BF16, tag="warm")
        nc.vector.memset(warm_t[:], 0.0)
        warm_ps = psum_pool.tile([128, 512], F32, tag="ps0_0", name="warm_ps")
        for i in range(N_WARM):
            nc.tensor.matmul(warm_ps[:], warm_t[:, 0:128],
                             warm_t[:, 128:640], start=True, stop=True)

        # x leads both DMA rings (tanh gates everything); bias rides last
        # (first needed at the degree-4 close, ~45us in)
        x_next, t1_next, t81_next = prep_head(0)
        w8d1_next = load_w8(0, 1)
        bias_t = const_pool.tile([128, O_], F32, tag="biasrep")
        nc.gpsimd.dma_start(out=bias_t[:], in_=biasrep)

        for c in range(n_chunks):
            b0 = c * b_chunk
            x_t, t1, t8_1 = x_next, t1_next, t81_next

            rings = [chain_pool.tile([128, FD], F32, tag=f"r{r}", name=f"r{r}_c{c}")
                     for r in range(3)]
            # stage doubles as group-1 staging and group-2 eviction buffer
            stage = stage_pool.tile([128, MT * OHT * 512], F32, tag="stage",
                                    name=f"st_c{c}")
            ps = [[psum_pool.tile([128, 512], F32, tag=f"ps{m}_{oh}",
                                  name=f"ps{m}_{oh}_c{c}")
                   for oh in range(OHT)] for m in range(MT)]

            # chunk 0 has no previous-chunk window to amortize weight DMA, so
            # its sync ring is oversubscribed; route degrees 3/4 via the
            # gpsimd ring there (allocation order d1..d4 preserved for pool
            # buffer rotation)
            w8_d = {1: w8d1_next, 2: load_w8(c, 2)}
            if c == 0:
                w8_d[3] = load_w8(c, 3, eng=nc.gpsimd)
                if d4kp > 0:
                    w8_d[4] = load_w8(c, 4, eng=nc.gpsimd)
            wb4_t = load_wb4(c)
            w_d5 = load_w(c, 5, eng=nc.gpsimd)

            def rec_half(cur, prev1, prev2, h):
                """T_next = 2*t1*prev1 - prev2 on k-half h (DVE multiply +
                in-place scalar_tensor_tensor)."""
                hs = slice(h * (FD // NH), (h + 1) * (FD // NH))
                nc.vector.tensor_tensor(cur[:, hs], t1[:, hs], prev1[:, hs], MULT)
                nc.vector.scalar_tensor_tensor(
                    cur[:, hs], cur[:, hs], 2.0, prev2[:, hs], MULT, SUBTRACT)

            def dbl_half(cur, src, h):
                """T_2n = 2*T_n^2 - 1 on k-half h: Square on ScalarE (keeps
                the DVE free in the fp8 window) + in-place DVE axpb."""
                hs = slice(h * (FD // NH), (h + 1) * (FD // NH))
                nc.scalar.activation(cur[:, hs], src[:, hs],
                                     mybir.ActivationFunctionType.Square)
                nc.vector.tensor_scalar(cur[:, hs], cur[:, hs],
                                        2.0, -1.0, MULT, ADD)

            def conv8_half(dst, src, h, nkp):
                """fp8 copy of k-half h into the k-major 4D pair tile."""
                kp0, kp1 = h * (KT // (2 * NH)), min(nkp, (h + 1) * (KT // (2 * NH)))
                if kp0 >= kp1:
                    return
                nc.scalar.activation(
                    dst[:, kp0:kp1, :, :],
                    src[:, kp0 * 2 * b_chunk:kp1 * 2 * b_chunk], COPY)

            # ---- group A: fp8 DoubleRow degrees 1..3 (+ partial 4) ----
            t8_t = {1: t8_1}
            t_prev2, t_prev1 = None, t1
            for d in (1, 2, 3):
                if d == 1 and 3 not in w8_d:
                    w8_d[3] = load_w8(c, 3)
                elif d == 2 and d4kp > 0 and 4 not in w8_d:
                    w8_d[4] = load_w8(c, 4)
                # recurrence + conversion for the NEXT degree, half by half
                nd = d + 1
                cur = rings[(nd - 2) % 3]
                t8_n = None
                if nd <= 3:
                    t8_n = t8_pool.tile([128, KP, 2, b_chunk], FP8, tag="t8",
                                        name=f"t8d{nd}_c{c}")
                elif d4kp > 0:
                    t8_n = t8b_pool.tile([128, d4kp, 2, b_chunk], FP8,
                                         tag="t8b", name=f"t8d4_c{c}")
                for h in range(NH):
                    if nd == 2:
                        dbl_half(cur, t1, h)          # T2 = 2*T1^2 - 1
                    elif nd == 4:
                        dbl_half(cur, rings[0], h)    # T4 = 2*T2^2 - 1
                    else:
                        rec_half(cur, t_prev1, t_prev2, h)
                    if t8_n is not None:
                        conv8_half(t8_n, cur, h, KP if nd <= 3 else d4kp)
                t_prev2, t_prev1 = t_prev1, cur
                if t8_n is not None:
                    t8_t[nd] = t8_n
                # this degree's matmuls (k-outer: stream as halves finish)
                closing = (d == 3 and d4kp == 0)
                if not closing:
                    for kp in range(KP):
                        for m in range(MT):
                            lhsT = t8_t[d][:, kp, :, m * 128:(m + 1) * 128]
                            for oh in range(OHT):
                                nc.tensor.matmul(
                                    ps[m][oh][:], lhsT,
                                    w8_d[d][kp][:, :, oh * 512:(oh + 1) * 512],
                                    start=(d == 1 and kp == 0), stop=False,
                                    perf_mode=DR)

            # degree 4 bf16 leftover conversion
            cur4 = t_prev1
            tb4 = None
            if nb4 > 0:
                tb4 = bb_pool.tile([128, nb4 * b_chunk], BF16, tag="tb4",
                                   name=f"tb4_c{c}")
                nc.scalar.activation(
                    tb4[:], cur4[:, 2 * d4kp * b_chunk:KT * b_chunk], COPY)

            # close group A per bank (k-contiguous) and stage the rescaled
            # partial + bias; banks free up one by one for the bf16 group
            close_d = 4 if d4kp > 0 else 3
            close_t8 = t8_t[close_d]
            close_w8 = w8_d[close_d]
            close_kp = KP if close_d == 3 else d4kp
            for m in range(MT):
                for oh in range(OHT):
                    for kp in range(close_kp):
                        lhsT = close_t8[:, kp, :, m * 128:(m + 1) * 128]
                        nc.tensor.matmul(
                            ps[m][oh][:], lhsT,
                            close_w8[kp][:, :, oh * 512:(oh + 1) * 512],
                            start=False, stop=(kp == close_kp - 1),
                            perf_mode=DR)
                    so = (m * OHT + oh) * 512
                    nc.vector.scalar_tensor_tensor(
                        stage[:, so:so + 512], ps[m][oh][:],
                        1.0 / W8_SCALE, bias_t[:, oh * 512:(oh + 1) * 512],
                        MULT, ADD)

            # ---- group B: bf16 degrees (rest of 4, then 5..8) ----
            if nb4 > 0:
                for j in range(nb4):
                    for m in range(MT):
                        lhsT = tb4[:, j * b_chunk + m * 128:
                                   j * b_chunk + (m + 1) * 128]
                        for oh in range(OHT):
                            nc.tensor.matmul(
                                ps[m][oh][:], lhsT,
                                wb4_t[j][:, oh * 512:(oh + 1) * 512],
                                start=(j == 0), stop=False)

            w_next_hi = w_d5
            for d in range(5, DEG + 1):
                w_t = w_next_hi
                # recurrence + bf16 conversion for this degree
                cur = rings[(d - 2) % 3]
                for h in range(NH):
                    rec_half(cur, t_prev1, t_prev2, h)
                t_prev2, t_prev1 = t_prev1, cur
                tb = bb_pool.tile([128, FD], BF16, tag="bb", name=f"tb{d}_c{c}")
                for q in range(4):
                    qs = slice(q * (FD // 4), (q + 1) * (FD // 4))
                    nc.scalar.activation(tb[:, qs], cur[:, qs], COPY)
                if d < DEG:
                    # degrees 6/8 ride sync, degree 7 rides the SWDGE
                    w_next_hi = load_w(c, d + 1,
                                       eng=nc.gpsimd if d == 6 else nc.sync)

                if d < DEG:
                    start_b = (d == 5 and nb4 == 0)
                    for k in range(KT):
                        for m in range(MT):
                            lhsT = tb[:, k * b_chunk + m * 128:
                                      k * b_chunk + (m + 1) * 128]
                            for oh in range(OHT):
                                nc.tensor.matmul(
                                    ps[m][oh][:], lhsT,
                                    w_t[k][:, oh * 512:(oh + 1) * 512],
                                    start=(start_b and k == 0), stop=False)
                else:
                    if c + 1 < n_chunks:
                        x_next, t1_next, t81_next = prep_head(c + 1)
                        w8d1_next = load_w8(c + 1, 1)
                    # close group B per bank; combine with staged group-A
                    # partial in-place and DMA out, bank by bank
                    for m in range(MT):
                        for oh in range(OHT):
                            for k in range(KT):
                                lhsT = tb[:, k * b_chunk + m * 128:
                                          k * b_chunk + (m + 1) * 128]
                                nc.tensor.matmul(
                                    ps[m][oh][:], lhsT,
                                    w_t[k][:, oh * 512:(oh + 1) * 512],
                                    start=False, stop=(k == KT - 1))
                            so = (m * OHT + oh) * 512
                            nc.vector.tensor_tensor(
                                stage[:, so:so + 512], ps[m][oh][:],
                                stage[:, so:so + 512], ADD)
                            nc.gpsimd.dma_start(
                                out=y[b0 + m * 128: b0 + (m + 1) * 128,
                                      oh * 512:(oh + 1) * 512],
                                in_=stage[:, so:so + 512])
    nc.compile()
    return nc


_NC_CACHE = {}


def _install_ntff_hook():
    """Provide antenv.axon_hooks (missing in this image) so trace=True works."""
    import sys
    import types
    if "antenv.axon_hooks" in sys.modules:
        return
    hook = None
    try:
        from trn_agent_boot.trn_boot import _ntff_profile_via_ctypes
        hook = _ntff_profile_via_ctypes("/opt/axon/libaxon_pjrt.so")
    except Exception:
        pass
    mod = types.ModuleType("antenv.axon_hooks")
    mod.get_axon_ntff_profile_hook = lambda: hook
    sys.modules["antenv.axon_hooks"] = mod
    # no remote artifact bucket in this container
    import concourse.bass_utils as _bu
    _bu.upload_artifacts = lambda tmpdir: tmpdir


def _prep_inputs(x, cheby_coeffs, b_shard=B_SHARD, n_cores=N_CORES):
    coeffs = np.asarray(cheby_coeffs, dtype=np.float32)
    I_ = coeffs.shape[0]
    O_ = coeffs.shape[1]
    KT, KP, d4kp, nb4 = _cfg(I_)
    wmoved = np.moveaxis(coeffs[:, :, 1:], 2, 0)      # (DEG, I, O)

    # fp8 pair-tiles: degrees 1..3 all k-pairs, degree 4 first d4kp pairs.
    # layout [tile, partition, j, o] matches the [128, 2, O] SBUF tile.
    NT8 = 3 * KP + d4kp
    w8p = np.empty((NT8, 128, 2, O_), dtype=np.float32)
    for d in (1, 2, 3, 4):
        nkp = d4kp if d == 4 else KP
        for kp in range(nkp):
            f = 3 * KP + kp if d == 4 else (d - 1) * KP + kp
            for j in range(2):
                sl = wmoved[d - 1, (2 * kp + j) * 128:(2 * kp + j + 1) * 128, :]
                w8p[f, :, j, :] = sl
    w8p = np.ascontiguousarray(w8p * W8_SCALE).astype(ml_dtypes.float8_e4m3)

    wb4 = np.ascontiguousarray(
        wmoved[3, 2 * d4kp * 128:, :].reshape(nb4, 128, O_)
    ).astype(ml_dtypes.bfloat16)
    whi = np.ascontiguousarray(wmoved[4:]).astype(ml_dtypes.bfloat16)
    bias = coeffs[:, :, 0].astype(np.float64).sum(axis=0).astype(np.float32)
    biasrep = np.ascontiguousarray(np.broadcast_to(bias, (128, O_)))
    xT = np.asarray(x, dtype=np.float32).T  # (I, B)
    in_maps = []
    for c in range(n_cores):
        in_maps.append({
            "xT": np.ascontiguousarray(xT[:, c * b_shard:(c + 1) * b_shard]),
            "w8p": w8p,
            "wb4": wb4,
            "whi": whi,
            "biasrep": biasrep,
        })
    return in_maps


def kernel(x: np.ndarray, cheby_coeffs: np.ndarray, _trace: bool = False):
    assert x.shape == (B, I) and cheby_coeffs.shape == (I, O, DEG + 1)
    if _trace:
        _install_ntff_hook()
    if "nc" not in _NC_CACHE:
        _NC_CACHE["nc"] = build_nc()
    nc = _NC_CACHE["nc"]

    in_maps = _prep_inputs(x, cheby_coeffs)
    res = run_bass_kernel_spmd(nc, in_maps, list(range(N_CORES)), trace=_trace)
    out = np.concatenate([res.results[c]["y"] for c in range(N_CORES)], axis=0)
    if _trace:
        return out, res
    return out
